# revision 1
# baseline (speedup 1.0000x reference)
"""Contrastive-loss (InfoNCE re-rank) Trainium2 Bass kernel.

Full op: q,k [256,1024], neg [256,2048,1024] f32.
  l_pos[n]   = q[n].k[n]
  l_neg[n,j] = neg[n,j].q[n]
  loss = mean_n( LSE(logits_n/T) - l_pos[n]/T ),  T = 0.07

Sharding: data-parallel over N across 8 NeuronCores (32 samples/core).
Each core streams its 268 MB neg shard once (memory-bound), computes the
per-sample scaled logits with fused DVE tensor_tensor_reduce, a two-level
log-sum-exp (free-dim level on DVE/ACT, partition level via PE transpose),
and writes 32 per-sample NLLs. Host gathers 8x32 NLLs and takes the mean.
"""

import numpy as np

import concourse.bass as bass
import concourse.bacc as bacc
import concourse.tile as tile
from concourse import mybir
from concourse.masks import make_identity
from concourse.bass_utils import run_bass_kernel_spmd

N, C, K = 256, 1024, 2048
NCORES = 8
NLOC = N // NCORES          # 32 samples per core
P = 128                     # SBUF partitions
TJ = K // P                 # 16 j-tiles of 128 negatives per sample
TCH = 4                     # j-tiles per DMA chunk (2 MB per dma_start)
TEMP = 0.07
SCALE = 1.0 / TEMP
F32 = mybir.dt.float32
ALU = mybir.AluOpType
ACT = mybir.ActivationFunctionType


def build_module() -> bass.Bass:
    # Bacc (not plain Bass): its compile() runs generate_event_semaphores,
    # which splits multi-sem waits into separate event instructions — this
    # walrus rejects >1 sync wait per instruction.
    nc = bacc.Bacc("TRN2", target_bir_lowering=False)
    q_d = nc.dram_tensor("q", [NLOC, C], F32, kind="ExternalInput")
    k_d = nc.dram_tensor("k", [NLOC, C], F32, kind="ExternalInput")
    neg_d = nc.dram_tensor("neg", [NLOC, K, C], F32, kind="ExternalInput")
    out_d = nc.dram_tensor("nll", [NLOC, 1], F32, kind="ExternalOutput")

    # neg[n, t*P + p, c] viewed as [n, p, t, c] so partition dim is the
    # inner 128 of the negative index and each partition row is a
    # contiguous 4 KB run in HBM.
    neg_r = neg_d[:].rearrange("n (t p) c -> n p t c", p=P)

    with tile.TileContext(nc) as tc:
        with (
            tc.tile_pool(name="consts", bufs=1) as consts,
            tc.tile_pool(name="small", bufs=1) as small,
            tc.tile_pool(name="qrow", bufs=3) as qrow_pool,
            tc.tile_pool(name="qb", bufs=3) as qb_pool,
            tc.tile_pool(name="negp", bufs=6) as negp,
            tc.tile_pool(name="ps", bufs=1, space="PSUM") as ps_pool,
        ):
            identity = consts.tile([P, P], F32)
            make_identity(nc, identity)

            q_sb = consts.tile([NLOC, C], F32)
            k_sb = consts.tile([NLOC, C], F32)
            nc.sync.dma_start(out=q_sb, in_=q_d[:])
            nc.sync.dma_start(out=k_sb, in_=k_d[:])

            # y_pos[n] = (q[n].k[n]) / T, one fused mult+reduce.
            pos_scr = small.tile([NLOC, C], F32)
            ypos = small.tile([NLOC, 1], F32)
            nc.vector.scalar_tensor_tensor(
                out=pos_scr, in0=q_sb, scalar=SCALE, in1=k_sb,
                op0=ALU.mult, op1=ALU.mult, accum_out=ypos,
            )

            # Scaled negative logits: Y[p, n, t] = neg[n, t*P+p].q[n] / T
            Y = small.tile([P, NLOC, TJ], F32)
            ttr_scr = small.tile([P, C], F32)  # discarded elementwise product

            for n in range(NLOC):
                # partition_broadcast's source must start at partition 0,
                # so stage this sample's q row there first.
                q_row = qrow_pool.tile([1, C], F32)
                nc.sync.dma_start(out=q_row, in_=q_d[n : n + 1, :])
                q_b = qb_pool.tile([P, C], F32)
                q_row_ap = q_row[:]
                src = bass.AP(
                    tensor=q_row_ap.tensor,
                    offset=q_row_ap.offset,
                    # partition dim stays count-1 (nonzero step); the
                    # 128-way replication rides a 0-step free dim.
                    ap=[[1, 1], [0, P], [1, C]],
                )
                nc.sync.dma_start(out=q_b, in_=src)
                for ch in range(TJ // TCH):
                    neg_t = negp.tile([P, TCH, C], F32)
                    nc.sync.dma_start(
                        out=neg_t,
                        in_=neg_r[n, :, ch * TCH : (ch + 1) * TCH, :],
                    )
                    for t in range(TCH):
                        tt = ch * TCH + t
                        nc.vector.scalar_tensor_tensor(
                            out=ttr_scr, in0=neg_t[:, t, :], scalar=SCALE,
                            in1=q_b, op0=ALU.mult, op1=ALU.mult,
                            accum_out=Y[:, n, tt : tt + 1],
                        )

            # Level-1 LSE along the free dim: lse_p[p, n] over 16 values.
            m_all = small.tile([P, NLOC], F32)
            nc.vector.reduce_max(out=m_all, in_=Y, axis=mybir.AxisListType.X)
            mneg = small.tile([P, NLOC], F32)
            nc.scalar.mul(mneg, m_all, -1.0)

            s_all = small.tile([P, NLOC], F32)
            exp_scr = small.tile([P, TJ], F32)
            for n in range(NLOC):
                nc.scalar.activation(
                    out=exp_scr, in_=Y[:, n, :], func=ACT.Exp,
                    bias=mneg[:, n : n + 1], scale=1.0,
                    accum_out=s_all[:, n : n + 1],
                )
            lse_p = small.tile([P, NLOC], F32)
            nc.scalar.activation(out=lse_p, in_=s_all, func=ACT.Ln)
            nc.vector.tensor_add(out=lse_p, in0=lse_p, in1=m_all)

            # Level-2 LSE across partitions: transpose so samples sit on
            # partitions, append y_pos as the 129th "group".
            pt = ps_pool.tile([NLOC, P], F32)
            nc.tensor.transpose(out=pt, in_=lse_p, identity=identity)
            lt = small.tile([NLOC, P + 1], F32)
            nc.scalar.copy(out=lt[:, :P], in_=pt)
            nc.vector.tensor_copy(out=lt[:, P : P + 1], in_=ypos)

            m2 = small.tile([NLOC, 1], F32)
            nc.vector.reduce_max(out=m2, in_=lt, axis=mybir.AxisListType.X)
            m2neg = small.tile([NLOC, 1], F32)
            nc.scalar.mul(m2neg, m2, -1.0)
            e2_scr = small.tile([NLOC, P + 1], F32)
            s2 = small.tile([NLOC, 1], F32)
            nc.scalar.activation(
                out=e2_scr, in_=lt, func=ACT.Exp,
                bias=m2neg, scale=1.0, accum_out=s2,
            )
            ln_s2 = small.tile([NLOC, 1], F32)
            nc.scalar.activation(out=ln_s2, in_=s2, func=ACT.Ln)

            # nll[n] = (m2 + ln s2) - y_pos[n]
            nll = small.tile([NLOC, 1], F32)
            nc.vector.tensor_scalar(
                out=nll, in0=ln_s2, scalar1=m2, scalar2=ypos,
                op0=ALU.add, op1=ALU.subtract,
            )
            nc.sync.dma_start(out=out_d[:], in_=nll)

    nc.finalize()
    return nc


_CACHED = {}


def _run(q, k, neg, trace=False):
    if "nc" not in _CACHED:
        _CACHED["nc"] = build_module()
    nc = _CACHED["nc"]
    in_maps = []
    for c in range(NCORES):
        s = slice(c * NLOC, (c + 1) * NLOC)
        in_maps.append({"q": q[s], "k": k[s], "neg": neg[s]})
    res = run_bass_kernel_spmd(
        nc, in_maps, core_ids=list(range(NCORES)), trace=trace
    )
    nll = np.concatenate([r["nll"].reshape(-1) for r in res.results])
    loss = np.asarray(np.mean(nll.astype(np.float64)), dtype=np.float32)
    return loss, res


def kernel(q, k, neg):
    q = np.ascontiguousarray(np.asarray(q, dtype=np.float32))
    k = np.ascontiguousarray(np.asarray(k, dtype=np.float32))
    neg = np.ascontiguousarray(np.asarray(neg, dtype=np.float32))
    loss, _ = _run(q, k, neg, trace=False)
    return loss



# revision 8
# speedup vs baseline: 1.4723x; 1.4723x over previous
"""Contrastive-loss (InfoNCE re-rank) Trainium2 Bass kernel.

Full op: q,k [256,1024], neg [256,2048,1024] f32.
  l_pos[n]   = q[n].k[n]
  l_neg[n,j] = neg[n,j].q[n]
  loss = mean_n( LSE(logits_n/T) - l_pos[n]/T ),  T = 0.07

Sharding: data-parallel over N across 8 NeuronCores (32 samples/core).

v2 layout/engine plan (vs v1 baseline @1.25ms):
 - neg viewed as [n, p, (t c)] with j = p*16 + t, so each partition's
   slice of one sample is 64 KB contiguous in HBM. DMAs are [128, 8192]
   f32 (4 MB, 32 KB/partition runs) instead of 2 MB chunks of scattered
   4 KB lines. LSE is permutation-invariant so the j relabeling is free.
 - q[n] broadcast to 128 partitions via PE ones-matmul into PSUM (idle
   engine) instead of 0-stride HBM DMA reads; the DVE dot-product
   instructions read q directly from PSUM. Kills all broadcast DMA
   traffic + queue serialization (the v1 bottleneck).
 - per-(n,t) scaled dot products stay fused scalar_tensor_tensor on DVE
   (~637us total), hidden under the ~750us HBM-roofline neg stream.
"""

import numpy as np

import concourse.bass as bass
import concourse.bacc as bacc
import concourse.tile as tile
from concourse import mybir
from concourse.masks import make_identity
from concourse.bass_utils import run_bass_kernel_spmd

N, C, K = 256, 1024, 2048
NCORES = 8
NLOC = N // NCORES          # 32 samples per core
P = 128                     # SBUF partitions
TJ = K // P                 # 16 negatives per partition per sample
HT = TJ // 2                # 8 per half-sample DMA tile
HTC = HT * C                # 8192 f32 per partition per tile
TEMP = 0.07
SCALE = 1.0 / TEMP
F32 = mybir.dt.float32
ALU = mybir.AluOpType
ACT = mybir.ActivationFunctionType


def build_module() -> bass.Bass:
    # Bacc (not plain Bass): its compile() runs generate_event_semaphores,
    # which splits multi-sem waits into separate event instructions — this
    # walrus rejects >1 sync wait per instruction.
    nc = bacc.Bacc("TRN2", target_bir_lowering=False)
    q_d = nc.dram_tensor("q", [NLOC, C], F32, kind="ExternalInput")
    k_d = nc.dram_tensor("k", [NLOC, C], F32, kind="ExternalInput")
    neg_d = nc.dram_tensor("neg", [NLOC, K, C], F32, kind="ExternalInput")
    # host-built one-hot stationaries (eye(NLOC) with columns repeated P
    # times): sel[:, n*P:(n+1)*P].T @ q_sb broadcasts q[n] to 128 partitions
    # on the PE. (Built on host: engine memsets can't start at partition>0.)
    sel_d = nc.dram_tensor("sel", [NLOC, NLOC * P], F32, kind="ExternalInput")
    out_d = nc.dram_tensor("nll", [NLOC, 1], F32, kind="ExternalOutput")

    # neg[n, p*16 + t, c] viewed as [n, p, t*C + c]: each partition's
    # 16 negatives are 16 KiB*4 contiguous in HBM -> large descriptors.
    neg_r = neg_d[:].rearrange("n (p t) c -> n p (t c)", p=P)

    with tile.TileContext(nc) as tc:
        with (
            tc.tile_pool(name="consts", bufs=1) as consts,
            tc.tile_pool(name="small", bufs=1) as small,
            tc.tile_pool(name="negp", bufs=4) as negp,
            tc.tile_pool(name="psq", bufs=3, space="PSUM") as psq,
            tc.tile_pool(name="pst", bufs=1, space="PSUM") as pst,
        ):
            identity = consts.tile([P, P], F32)
            make_identity(nc, identity)

            sel = consts.tile([NLOC, NLOC * P], F32)
            nc.sync.dma_start(out=sel, in_=sel_d[:])

            q_sb = consts.tile([NLOC, C], F32)
            k_sb = consts.tile([NLOC, C], F32)
            nc.sync.dma_start(out=q_sb, in_=q_d[:])
            nc.sync.dma_start(out=k_sb, in_=k_d[:])

            # y_pos[n] = (q[n].k[n]) / T, one fused mult+reduce.
            pos_scr = small.tile([NLOC, C], F32)
            ypos = small.tile([NLOC, 1], F32)
            nc.vector.scalar_tensor_tensor(
                out=pos_scr, in0=q_sb, scalar=SCALE, in1=k_sb,
                op0=ALU.mult, op1=ALU.mult, accum_out=ypos,
            )

            # Scaled negative logits: Y[p, n, t] = neg[n, p*16+t].q[n] / T
            Y = small.tile([P, NLOC, TJ], F32)
            ttr_scr = small.tile([P, C], F32)  # discarded elementwise product

            for n in range(NLOC):
                # Broadcast q[n] to all 128 partitions on the (idle) PE:
                # ones[1,128].T @ q[n][1,1024] -> PSUM [128, 1024].
                q_b = psq.tile([P, C], F32)
                # two matmuls: a single one may not span PSUM banks (512 f32)
                for mh in range(2):
                    nc.tensor.matmul(
                        q_b[:, mh * 512 : (mh + 1) * 512],
                        sel[:, n * P : (n + 1) * P],
                        q_sb[:, mh * 512 : (mh + 1) * 512],
                        start=True, stop=True,
                    )
                for h in range(2):
                    neg_t = negp.tile([P, HTC], F32)
                    nc.sync.dma_start(
                        out=neg_t,
                        in_=neg_r[n, :, h * HTC : (h + 1) * HTC],
                    )
                    for t in range(HT):
                        tt = h * HT + t
                        nc.vector.scalar_tensor_tensor(
                            out=ttr_scr,
                            in0=neg_t[:, t * C : (t + 1) * C],
                            scalar=SCALE,
                            in1=q_b,
                            op0=ALU.mult, op1=ALU.mult,
                            accum_out=Y[:, n, tt : tt + 1],
                        )

            # Level-1 LSE along the free dim: lse_p[p, n] over 16 values.
            m_all = small.tile([P, NLOC], F32)
            nc.vector.reduce_max(out=m_all, in_=Y, axis=mybir.AxisListType.X)
            mneg = small.tile([P, NLOC], F32)
            nc.scalar.mul(mneg, m_all, -1.0)

            s_all = small.tile([P, NLOC], F32)
            exp_scr = small.tile([P, TJ], F32)
            for n in range(NLOC):
                nc.scalar.activation(
                    out=exp_scr, in_=Y[:, n, :], func=ACT.Exp,
                    bias=mneg[:, n : n + 1], scale=1.0,
                    accum_out=s_all[:, n : n + 1],
                )
            lse_p = small.tile([P, NLOC], F32)
            nc.scalar.activation(out=lse_p, in_=s_all, func=ACT.Ln)
            nc.vector.tensor_add(out=lse_p, in0=lse_p, in1=m_all)

            # Level-2 LSE across partitions: transpose so samples sit on
            # partitions, append y_pos as the 129th "group".
            pt = pst.tile([NLOC, P], F32)
            nc.tensor.transpose(out=pt, in_=lse_p, identity=identity)
            lt = small.tile([NLOC, P + 1], F32)
            nc.scalar.copy(out=lt[:, :P], in_=pt)
            nc.vector.tensor_copy(out=lt[:, P : P + 1], in_=ypos)

            m2 = small.tile([NLOC, 1], F32)
            nc.vector.reduce_max(out=m2, in_=lt, axis=mybir.AxisListType.X)
            m2neg = small.tile([NLOC, 1], F32)
            nc.scalar.mul(m2neg, m2, -1.0)
            e2_scr = small.tile([NLOC, P + 1], F32)
            s2 = small.tile([NLOC, 1], F32)
            nc.scalar.activation(
                out=e2_scr, in_=lt, func=ACT.Exp,
                bias=m2neg, scale=1.0, accum_out=s2,
            )
            ln_s2 = small.tile([NLOC, 1], F32)
            nc.scalar.activation(out=ln_s2, in_=s2, func=ACT.Ln)

            # nll[n] = (m2 + ln s2) - y_pos[n]
            nll = small.tile([NLOC, 1], F32)
            nc.vector.tensor_scalar(
                out=nll, in0=ln_s2, scalar1=m2, scalar2=ypos,
                op0=ALU.add, op1=ALU.subtract,
            )
            nc.sync.dma_start(out=out_d[:], in_=nll)

    nc.finalize()
    return nc


_CACHED = {}


def _run(q, k, neg, trace=False):
    if "nc" not in _CACHED:
        _CACHED["nc"] = build_module()
    nc = _CACHED["nc"]
    sel = np.repeat(np.eye(NLOC, dtype=np.float32), P, axis=1)
    in_maps = []
    for c in range(NCORES):
        s = slice(c * NLOC, (c + 1) * NLOC)
        in_maps.append({"q": q[s], "k": k[s], "neg": neg[s], "sel": sel})
    res = run_bass_kernel_spmd(
        nc, in_maps, core_ids=list(range(NCORES)), trace=trace
    )
    nll = np.concatenate([r["nll"].reshape(-1) for r in res.results])
    loss = np.asarray(np.mean(nll.astype(np.float64)), dtype=np.float32)
    return loss, res


def kernel(q, k, neg):
    q = np.ascontiguousarray(np.asarray(q, dtype=np.float32))
    k = np.ascontiguousarray(np.asarray(k, dtype=np.float32))
    neg = np.ascontiguousarray(np.asarray(neg, dtype=np.float32))
    loss, _ = _run(q, k, neg, trace=False)
    return loss


# revision 10
# speedup vs baseline: 1.7698x; 1.2021x over previous
"""Contrastive-loss (InfoNCE re-rank) Trainium2 Bass kernel.

Full op: q,k [256,1024], neg [256,2048,1024] f32.
  l_pos[n]   = q[n].k[n]
  l_neg[n,j] = neg[n,j].q[n]
  loss = mean_n( LSE(logits_n/T) - l_pos[n]/T ),  T = 0.07

Sharding: data-parallel over N across 8 NeuronCores (32 samples/core).

v2 layout/engine plan (vs v1 baseline @1.25ms):
 - neg viewed as [n, p, (t c)] with j = p*16 + t, so each partition's
   slice of one sample is 64 KB contiguous in HBM. DMAs are [128, 8192]
   f32 (4 MB, 32 KB/partition runs) instead of 2 MB chunks of scattered
   4 KB lines. LSE is permutation-invariant so the j relabeling is free.
 - q[n] broadcast to 128 partitions via PE ones-matmul into PSUM (idle
   engine) instead of 0-stride HBM DMA reads; the DVE dot-product
   instructions read q directly from PSUM. Kills all broadcast DMA
   traffic + queue serialization (the v1 bottleneck).
 - per-(n,t) scaled dot products stay fused scalar_tensor_tensor on DVE
   (~637us total), hidden under the ~750us HBM-roofline neg stream.
"""

import numpy as np

import concourse.bass as bass
import concourse.bacc as bacc
import concourse.tile as tile
from concourse import mybir
from concourse.masks import make_identity
from concourse.bass_utils import run_bass_kernel_spmd

N, C, K = 256, 1024, 2048
NCORES = 8
NLOC = N // NCORES          # 32 samples per core
P = 128                     # SBUF partitions
TJ = K // P                 # 16 negatives per partition per sample
HT = TJ // 2                # 8 per half-sample DMA tile
HTC = HT * C                # 8192 f32 per partition per tile
TEMP = 0.07
SCALE = 1.0 / TEMP
F32 = mybir.dt.float32
ALU = mybir.AluOpType
ACT = mybir.ActivationFunctionType


def build_module() -> bass.Bass:
    # Bacc (not plain Bass): its compile() runs generate_event_semaphores,
    # which splits multi-sem waits into separate event instructions — this
    # walrus rejects >1 sync wait per instruction.
    nc = bacc.Bacc("TRN2", target_bir_lowering=False)
    q_d = nc.dram_tensor("q", [NLOC, C], F32, kind="ExternalInput")
    k_d = nc.dram_tensor("k", [NLOC, C], F32, kind="ExternalInput")
    neg_d = nc.dram_tensor("neg", [NLOC, K, C], F32, kind="ExternalInput")
    # host-built one-hot stationaries (eye(NLOC) with columns repeated P
    # times): sel[:, n*P:(n+1)*P].T @ q_sb broadcasts q[n] to 128 partitions
    # on the PE. (Built on host: engine memsets can't start at partition>0.)
    sel_d = nc.dram_tensor("sel", [NLOC, NLOC * P], F32, kind="ExternalInput")
    out_d = nc.dram_tensor("nll", [NLOC, 1], F32, kind="ExternalOutput")

    # neg[n, p*16 + t, c] viewed as [n, p, t*C + c]: each partition's
    # 16 negatives are 16 KiB*4 contiguous in HBM -> large descriptors.
    neg_r = neg_d[:].rearrange("n (p t) c -> n p (t c)", p=P)

    with tile.TileContext(nc) as tc:
        with (
            tc.tile_pool(name="consts", bufs=1) as consts,
            tc.tile_pool(name="small", bufs=1) as small,
            tc.tile_pool(name="negp", bufs=5) as negp,
            tc.tile_pool(name="psq", bufs=3, space="PSUM") as psq,
            tc.tile_pool(name="pst", bufs=1, space="PSUM") as pst,
        ):
            identity = consts.tile([P, P], F32)
            make_identity(nc, identity)

            sel = consts.tile([NLOC, NLOC * P], F32)
            nc.sync.dma_start(out=sel, in_=sel_d[:])

            q_sb = consts.tile([NLOC, C], F32)
            k_sb = consts.tile([NLOC, C], F32)
            nc.sync.dma_start(out=q_sb, in_=q_d[:])
            nc.sync.dma_start(out=k_sb, in_=k_d[:])

            # y_pos[n] = (q[n].k[n]) / T, one fused mult+reduce.
            pos_scr = small.tile([NLOC, C], F32)
            ypos = small.tile([NLOC, 1], F32)
            nc.vector.scalar_tensor_tensor(
                out=pos_scr, in0=q_sb, scalar=SCALE, in1=k_sb,
                op0=ALU.mult, op1=ALU.mult, accum_out=ypos,
            )

            # Scaled negative logits: Y[p, n, t] = neg[n, p*16+t].q[n] / T
            Y = small.tile([P, NLOC, TJ], F32)
            ttr_scr = small.tile([P, C], F32)  # discarded elementwise product
            m_all = small.tile([P, NLOC], F32)
            mneg = small.tile([P, NLOC], F32)
            s_all = small.tile([P, NLOC], F32)
            exp_scr = small.tile([P, TJ], F32)

            # HWDGE has two physical rings (SP + ACT sequencers); alternate
            # so consecutive neg loads never queue behind each other's
            # issue/completion bookkeeping on one ring.
            dma_engines = (nc.sync, nc.scalar)

            for n in range(NLOC):
                # Broadcast q[n] to all 128 partitions on the (idle) PE:
                # sel_n.T @ q_sb -> PSUM [128, 1024].
                q_b = psq.tile([P, C], F32)
                # two matmuls: a single one may not span PSUM banks (512 f32)
                for mh in range(2):
                    nc.tensor.matmul(
                        q_b[:, mh * 512 : (mh + 1) * 512],
                        sel[:, n * P : (n + 1) * P],
                        q_sb[:, mh * 512 : (mh + 1) * 512],
                        start=True, stop=True,
                    )
                for h in range(2):
                    neg_t = negp.tile([P, HTC], F32)
                    dma_engines[(2 * n + h) % 2].dma_start(
                        out=neg_t,
                        in_=neg_r[n, :, h * HTC : (h + 1) * HTC],
                    )
                    for t in range(HT):
                        tt = h * HT + t
                        nc.vector.scalar_tensor_tensor(
                            out=ttr_scr,
                            in0=neg_t[:, t * C : (t + 1) * C],
                            scalar=SCALE,
                            in1=q_b,
                            op0=ALU.mult, op1=ALU.mult,
                            accum_out=Y[:, n, tt : tt + 1],
                        )
                # Level-1 LSE for this sample inline (overlaps the stream;
                # keeps the post-loop tail to the last sample only).
                nc.vector.reduce_max(
                    out=m_all[:, n : n + 1], in_=Y[:, n, :],
                    axis=mybir.AxisListType.X,
                )
                nc.scalar.mul(
                    mneg[:, n : n + 1], m_all[:, n : n + 1], -1.0
                )
                nc.scalar.activation(
                    out=exp_scr, in_=Y[:, n, :], func=ACT.Exp,
                    bias=mneg[:, n : n + 1], scale=1.0,
                    accum_out=s_all[:, n : n + 1],
                )

            lse_p = small.tile([P, NLOC], F32)
            nc.scalar.activation(out=lse_p, in_=s_all, func=ACT.Ln)
            nc.vector.tensor_add(out=lse_p, in0=lse_p, in1=m_all)

            # Level-2 LSE across partitions: transpose so samples sit on
            # partitions, append y_pos as the 129th "group".
            pt = pst.tile([NLOC, P], F32)
            nc.tensor.transpose(out=pt, in_=lse_p, identity=identity)
            lt = small.tile([NLOC, P + 1], F32)
            nc.scalar.copy(out=lt[:, :P], in_=pt)
            nc.vector.tensor_copy(out=lt[:, P : P + 1], in_=ypos)

            m2 = small.tile([NLOC, 1], F32)
            nc.vector.reduce_max(out=m2, in_=lt, axis=mybir.AxisListType.X)
            m2neg = small.tile([NLOC, 1], F32)
            nc.scalar.mul(m2neg, m2, -1.0)
            e2_scr = small.tile([NLOC, P + 1], F32)
            s2 = small.tile([NLOC, 1], F32)
            nc.scalar.activation(
                out=e2_scr, in_=lt, func=ACT.Exp,
                bias=m2neg, scale=1.0, accum_out=s2,
            )
            ln_s2 = small.tile([NLOC, 1], F32)
            nc.scalar.activation(out=ln_s2, in_=s2, func=ACT.Ln)

            # nll[n] = (m2 + ln s2) - y_pos[n]
            nll = small.tile([NLOC, 1], F32)
            nc.vector.tensor_scalar(
                out=nll, in0=ln_s2, scalar1=m2, scalar2=ypos,
                op0=ALU.add, op1=ALU.subtract,
            )
            nc.sync.dma_start(out=out_d[:], in_=nll)

    nc.finalize()
    return nc


_CACHED = {}


def _run(q, k, neg, trace=False):
    if "nc" not in _CACHED:
        _CACHED["nc"] = build_module()
    nc = _CACHED["nc"]
    sel = np.repeat(np.eye(NLOC, dtype=np.float32), P, axis=1)
    in_maps = []
    for c in range(NCORES):
        s = slice(c * NLOC, (c + 1) * NLOC)
        in_maps.append({"q": q[s], "k": k[s], "neg": neg[s], "sel": sel})
    res = run_bass_kernel_spmd(
        nc, in_maps, core_ids=list(range(NCORES)), trace=trace
    )
    nll = np.concatenate([r["nll"].reshape(-1) for r in res.results])
    loss = np.asarray(np.mean(nll.astype(np.float64)), dtype=np.float32)
    return loss, res


def kernel(q, k, neg):
    q = np.ascontiguousarray(np.asarray(q, dtype=np.float32))
    k = np.ascontiguousarray(np.asarray(k, dtype=np.float32))
    neg = np.ascontiguousarray(np.asarray(neg, dtype=np.float32))
    loss, _ = _run(q, k, neg, trace=False)
    return loss
